# revision 27
# baseline (speedup 1.0000x reference)
"""Trainium2 Bass kernel for the autoregressive GRU decoder.

Problem: 512 sequential GRU steps over batch 4096, hidden 128; per step the
3-dim position output feeds back into the input.  Data-parallel over 8 cores
(512 batch rows per core), with the per-core batch split into 2 streams of
256 so the engines can pipeline across the sequential dependency chain.

Measured on hardware: 2.14 ms for 512 steps (vs 2.77 ms for the previous
revision of this kernel); per-step period ~4.1 us, bounded by the serial
dependency chain MM -> sigmoid -> mult -> add -> tanh -> sub -> mult
across TensorE/ACT/DVE with two anti-phased streams sharing the engines.

Algebraic restructuring done on the host (validated vs fp64 golden):
  - pos_k = W_out h_k + b_out feeds the next step's input, so the input
    matmul folds into the hidden one: W_c = W_ih[:, :3] @ W_out + W_hh, with
    all z / bias contributions collapsed into one constant C per gate.
  - The gate pre-activations live persistently in PSUM.  They are
    initialized once with C (via an identity matmul) + W @ h, and every
    subsequent step only accumulates W_c @ delta where
    delta = h_new - h = (u - 1) * (h - n).  No per-step constant re-adds.
  - pos is delta-accumulated directly in OUTPUT layout: posT[128(b), 4c*3i]
    persistent in one PSUM bank, where chunk c covers batch rows c*128..;
    each step 4 tiny N=3 matmuls (delta chunk stationary, W_out^T moving)
    accumulate it, and ONE [128, 12] ACT copy snapshots it (303 ns vs 677
    for the untransposed [3, 512] layout).  No final transpose pass: the
    staging buffer DMAs straight into out[b, t, i].
  - h lives in a PSUM bank, delta-accumulated by per-stream identity
    matmuls on TensorE.  This removes GpSimd from the kernel entirely:
    GpSimd shares its SBUF port with VectorE, and its streaming was
    measured to slow concurrent 2-port DVE ops by up to 2x.

Layout per core: PSUM banks: rz_A, nx_A, rz_B, nx_B (gate preacts,
[r|z] / [xn|hn] halves), h [128, 512], posT [128, 12].
"""

import os
import numpy as np

B, H, LAT, IN = 4096, 128, 32, 3
NCORES = 8
BSH = B // NCORES          # 512 batch rows per core
NST = 2                    # streams per core
BST = BSH // NST           # 256 batch rows per stream

# "f32" | "f32r" | "bf16" for the per-step delta matmuls (gates + pos);
# the init matmuls always run exact f32.
MM_DTYPE = os.environ.get("KERNEL_MM_DTYPE", "bf16")
# elementwise working dtype for t/s/n/ru/dd tiles: "f32" | "bf16"
EW_DTYPE = os.environ.get("KERNEL_EW_DTYPE", "bf16")

_CACHE = {}


def _host_prep(context, z, W_ih, W_hh, b_ih, b_hh, W_out, b_out):
    """Fold weights/constants; build per-core input maps."""
    f32 = np.float32
    sl = [slice(0, H), slice(H, 2 * H), slice(2 * H, 3 * H)]  # r, z, n rows

    Wp = (W_ih[:, :IN] @ W_out).astype(f32)           # pos feedback fold
    C0 = (W_ih[:, IN:] @ z.T + b_ih[:, None]).astype(f32)     # [384, B]
    C1 = (C0 + (W_ih[:, :IN] @ b_out)[:, None]).astype(f32)

    Wc_r = (Wp[sl[0]] + W_hh[sl[0]]).astype(f32)
    Wc_z = (Wp[sl[1]] + W_hh[sl[1]]).astype(f32)
    Wc_xn = Wp[sl[2]].astype(f32)
    W_hn = W_hh[sl[2]].astype(f32)

    def asc(a):
        return np.ascontiguousarray(a, dtype=f32)

    weights = {
        "w_r": asc(Wc_r.T), "w_z": asc(-Wc_z.T),
        "w_xn": asc(Wc_xn.T), "w_hn": asc(W_hn.T),
        "w0_r": asc(W_hh[sl[0]].T), "w0_z": asc(-W_hh[sl[1]].T),
        "w_out_t": asc(W_out.T),                       # [128, 3]
        "ident": np.eye(H, dtype=f32),
        "c_posT": asc(np.broadcast_to(np.tile(b_out, BSH // 128)[None, :],
                                      (H, (BSH // 128) * IN))),
    }

    in_maps = []
    for c in range(NCORES):
        bs = slice(c * BSH, (c + 1) * BSH)
        m = dict(weights)
        m["h0t"] = asc(context[bs].T)                  # [H, BSH]
        for X in range(NST):
            cs = slice(c * BSH + X * BST, c * BSH + (X + 1) * BST)
            bhh = b_hh[:, None]
            m[f"c0_rz_{X}"] = asc(np.concatenate(
                [C0[sl[0], cs] + bhh[sl[0]],
                 -(C0[sl[1], cs] + bhh[sl[1]])], axis=1))
            m[f"c1_rz_{X}"] = asc(np.concatenate(
                [C1[sl[0], cs] + bhh[sl[0]],
                 -(C1[sl[1], cs] + bhh[sl[1]])], axis=1))
            hn_const = np.broadcast_to(b_hh[sl[2]][:, None], (H, BST))
            m[f"c0_nx_{X}"] = asc(np.concatenate([C0[sl[2], cs], hn_const], axis=1))
            m[f"c1_nx_{X}"] = asc(np.concatenate([C1[sl[2], cs], hn_const], axis=1))
        in_maps.append(m)
    return in_maps


def _build(steps):
    import concourse.bacc as bacc
    import concourse.mybir as mybir
    from concourse.tile import TileContext

    f32 = mybir.dt.float32
    bf16 = mybir.dt.bfloat16
    Act = mybir.ActivationFunctionType
    Op = mybir.AluOpType

    ew_dt = bf16 if EW_DTYPE == "bf16" else f32
    # dtype for the per-step delta matmul operands (weights + delta).
    mm_dt = {"f32": f32, "f32r": mybir.dt.float32r, "bf16": bf16}[MM_DTYPE]

    nc = bacc.Bacc("TRN2", target_bir_lowering=False, debug=False)

    # ---- DRAM parameters ----
    names_2d = ["h0t"] + [f"c{i}_{g}_{X}" for i in (0, 1)
                          for g in ("rz", "nx") for X in range(NST)]
    params = {}
    for n in names_2d:
        params[n] = nc.declare_dram_parameter(n, [H, BSH], f32, isOutput=False)
    for n in ("w_r", "w_z", "w_xn", "w_hn", "w0_r", "w0_z", "ident"):
        params[n] = nc.declare_dram_parameter(n, [H, H], f32, isOutput=False)
    params["w_out_t"] = nc.declare_dram_parameter("w_out_t", [H, IN], f32,
                                                  isOutput=False)
    params["c_posT"] = nc.declare_dram_parameter(
        "c_posT", [H, (BSH // 128) * IN], f32, isOutput=False)
    p_out = nc.declare_dram_parameter("out", [BSH, steps, IN], f32,
                                      isOutput=True)
    NCH = BSH // 128           # batch chunks per core in the posT layout

    with TileContext(nc) as tc, \
            tc.tile_pool(name="const", bufs=1) as cpool, \
            tc.tile_pool(name="state", bufs=1) as spool, \
            tc.tile_pool(name="work", bufs=3) as wpool, \
            tc.tile_pool(name="psum", bufs=1, space="PSUM") as ppool:

        sb = {}
        for n, p in params.items():
            t = cpool.tile(list(p.shape), f32, name=f"sb_{n}")
            nc.sync.dma_start(out=t[:], in_=p[:])
            sb[n] = t

        # per-step matmul weights, converted to the mm dtype once
        stepw = {}
        for n in ("w_r", "w_z", "w_xn", "w_hn", "w_out_t", "ident"):
            if MM_DTYPE != "f32":
                t = cpool.tile(list(params[n].shape), mm_dt, name=f"cw_{n}")
                nc.vector.tensor_copy(t[:], sb[n][:])
                stepw[n] = t
            else:
                stepw[n] = sb[n]

        h = spool.tile([H, BSH], f32, name="h")
        nc.sync.dma_start(out=h[:], in_=params["h0t"][:])

        rz = [ppool.tile([128, 2 * BST], f32, name=f"ps_rz{X}")
              for X in range(NST)]
        nx = [ppool.tile([128, 2 * BST], f32, name=f"ps_nx{X}")
              for X in range(NST)]
        # hidden state, PSUM-resident, delta-accumulated by TensorE
        hps = ppool.tile([128, BSH], f32, name="ps_h")
        nc.tensor.matmul(hps[:], sb["ident"][:], h[:], start=True, stop=True)
        # pos in transposed/output layout: row p of chunk c is batch row
        # c*128+p; cols (c, i) hold pos[i].  Init: b_out + W_out h_0 (f32).
        posT = ppool.tile([128, NCH * IN], f32, name="ps_posT")
        # ONE start=True matmul for the whole region: start clears the
        # bank's has_written bits bank-wide, so per-chunk start=True inits
        # would wipe the previous chunks' accumulate state.
        nc.tensor.matmul(posT[:], sb["ident"][:], sb["c_posT"][:],
                         start=True, stop=False)
        for c in range(NCH):
            nc.tensor.matmul(posT[:, c * IN:(c + 1) * IN],
                             sb["h0t"][:, c * 128:(c + 1) * 128],
                             sb["w_out_t"][:], start=False, stop=True)

        delta_prev = [None, None]
        stagger = {}

        def emit_pos(dja):
            """posT += (delta chunk)^T @ W_out^T: 4 tiny N=3 matmuls with the
            delta chunk as the stationary operand; off the recurrence path."""
            for c in range(NCH):
                nc.tensor.matmul(posT[:, c * IN:(c + 1) * IN],
                                 dja[:, c * 128:(c + 1) * 128],
                                 stepw["w_out_t"][:], start=False, stop=True)

        TSNAP = 16                 # pos snapshots buffered in SBUF per DMA
        W = NCH * IN               # snapshot width (12 cols)
        stg = [None]

        def emit_snap(k):
            """Snapshot posT (state of step k) into the SBUF staging buffer
            (one cheap [128, 12] ACT copy, explicitly ordered after the
            chain-critical tanh so it never head-of-line blocks the ACT
            FIFO) and DMA a TSNAP-step block straight to out[b, t, i]."""
            kloc = k % TSNAP
            if kloc == 0:
                stg[0] = wpool.tile([128, TSNAP * W], f32, name="stg", bufs=2)
            # stg is laid out (c, t, i) so the flush DMA's DRAM AP fuses
            # (t i) into one contiguous dim (3-dim DMA limit).
            cp = nc.scalar.copy(
                stg[0][:].rearrange("p (c t i) -> p c t i",
                                    c=NCH, t=TSNAP)[:, :, kloc, :],
                posT[:].rearrange("p (c i) -> p c i", c=NCH))
            lt = stagger.get("last_tanh")
            if lt is not None:
                add_dep_helper(cp.ins, lt.ins, reason="snap after tanh")
            if kloc == TSNAP - 1:
                t0 = k - kloc
                nc.sync.dma_start(
                    out=p_out[:, t0:t0 + TSNAP, :].rearrange(
                        "(c p) t i -> p c (t i)", c=NCH),
                    in_=stg[0][:].rearrange("p (c ti) -> p c ti", c=NCH,
                                            ti=TSNAP * IN))

        from concourse.tile_rust import add_dep_helper

        def emit_gates(k, mode, X):
            """Gate matmuls + pos for one stream; returns the first matmul
            (for the anti-phase dependency ladder)."""
            first = None
            if mode < 2:
                ci = f"c{mode}"
                first = nc.tensor.matmul(rz[X][:], sb["ident"][:],
                                         sb[f"{ci}_rz_{X}"][:],
                                         start=True, stop=False)
                nc.tensor.matmul(nx[X][:], sb["ident"][:],
                                 sb[f"{ci}_nx_{X}"][:],
                                 start=True, stop=(mode == 0))
                wr = sb["w0_r"] if mode == 0 else sb["w_r"]
                wz = sb["w0_z"] if mode == 0 else sb["w_z"]
                gate_mms = [(wr, rz, 0, False), (wz, rz, 1, True),
                            (sb["w_hn"], nx, 1, True)]
                if mode == 1:
                    gate_mms.insert(2, (sb["w_xn"], nx, 0, False))
                for w, bank, half, stop in gate_mms:
                    nc.tensor.matmul(
                        bank[X][:, half * BST:(half + 1) * BST],
                        w[:], h[:, X * BST:(X + 1) * BST],
                        start=False, stop=stop)
            else:
                for w, bank, half, stop in [
                        (stepw["w_r"], rz, 0, False),
                        (stepw["w_z"], rz, 1, True),
                        (stepw["w_xn"], nx, 0, False),
                        (stepw["w_hn"], nx, 1, True)]:
                    mm = nc.tensor.matmul(
                        bank[X][:, half * BST:(half + 1) * BST],
                        w[:], delta_prev[X][:],
                        start=False, stop=stop)
                    if first is None:
                        first = mm
            return first

        for k in range(steps):
            mode = 0 if k == 0 else (1 if k == 1 else 2)

            # Anti-phase ladder: emit [gates_A, sigma_A, gates_B, sigma_B]
            # with gates_B additionally depending on sigma_A (same step) and
            # gates_A on sigma_B (previous step).  Without this the two
            # stream pipelines bunch up in phase and serialize on the shared
            # engines (bus-bunching); the ladder pins a half-step offset.
            ru = [wpool.tile([128, 2 * BST], ew_dt, name=f"ru{X}", bufs=4)
                  for X in range(NST)]
            tt = [wpool.tile([128, BST], ew_dt, name=f"t{X}", bufs=4)
                  for X in range(NST)]
            ts = [wpool.tile([128, BST], ew_dt, name=f"s{X}", bufs=4)
                  for X in range(NST)]
            old_delta = delta_prev
            dj = wpool.tile([128, BSH], mm_dt, name="dj", bufs=4)
            new_delta = [dj[:, 0:BST], dj[:, BST:2 * BST], dj]
            for X in range(NST):
                # ---- full per-stream chain: every engine's FIFO sees
                # [A-op, B-op, A-op, ...] so anti-phased streams never
                # head-of-line block each other. ----
                mm_first = emit_gates(k, mode, X)
                if mm_first is not None:
                    other = stagger.get("s_prev" if X == 0 else "s_cur")
                    if other is not None:
                        add_dep_helper(mm_first.ins, other.ins,
                                       reason="anti-phase ladder")
                # h <- h + delta(k-1) for this stream, as an identity-matmul
                # accumulate into the PSUM h bank.  Emitted here (one step
                # late, right behind this stream's gate matmuls) so it fills
                # the PE gap before the partner stream's gates become ready
                # and never head-of-line blocks a chain matmul.
                if old_delta[0] is not None:
                    nc.tensor.matmul(hps[:, X * BST:(X + 1) * BST],
                                     stepw["ident"][:], old_delta[X],
                                     start=False, stop=True)
                sig = nc.scalar.activation(ru[X][:], rz[X][:], Act.Sigmoid)
                stagger["s_cur" if X == 0 else "s_prev"] = sig
                nc.vector.tensor_tensor(tt[X][:], nx[X][:, BST:2 * BST],
                                        ru[X][:, 0:BST], Op.mult)
                nc.vector.tensor_tensor(ts[X][:], tt[X][:],
                                        nx[X][:, 0:BST], Op.add)
                n_ = wpool.tile([128, BST], ew_dt, name=f"n{X}", bufs=4)
                stagger["last_tanh"] = nc.scalar.activation(
                    n_[:], ts[X][:], Act.Tanh)
                dd = wpool.tile([128, BST], ew_dt, name=f"dd{X}", bufs=4)
                nc.vector.tensor_tensor(dd[:], n_[:],
                                        hps[:, X * BST:(X + 1) * BST],
                                        Op.subtract)
                delta_inst = nc.vector.tensor_tensor(new_delta[X],
                                                     ru[X][:, BST:2 * BST],
                                                     dd[:], Op.mult)
                stagger["last_delta"] = delta_inst
            delta_prev = new_delta

            if k == 0:
                # SBUF h copy is still the matmul rhs for the k=1 bootstrap
                # step; bring it to h_1 once (DVE, off-chain).
                nc.vector.tensor_tensor(h[:], h[:], new_delta[2][:], Op.add)

            # pos matmuls + snapshot for the previous step (off-chain tail)
            if old_delta[0] is not None:
                emit_pos(old_delta[2])
                emit_snap(k - 1)

        # trailing pos output for the final step
        emit_pos(delta_prev[2])
        emit_snap(steps - 1)

    nc.finalize()
    return nc


def _get_nc(steps):
    key = (steps, MM_DTYPE, EW_DTYPE)
    if key not in _CACHE:
        _CACHE[key] = _build(steps)
    return _CACHE[key]


def kernel(context, z, steps, W_ih, W_hh, b_ih, b_hh, W_out, b_out):
    from concourse.bass_utils import run_bass_kernel_spmd

    context = np.asarray(context, dtype=np.float32)
    z = np.asarray(z, dtype=np.float32)
    W_ih = np.asarray(W_ih, dtype=np.float32)
    W_hh = np.asarray(W_hh, dtype=np.float32)
    b_ih = np.asarray(b_ih, dtype=np.float32)
    b_hh = np.asarray(b_hh, dtype=np.float32)
    W_out = np.asarray(W_out, dtype=np.float32)
    b_out = np.asarray(b_out, dtype=np.float32)
    steps = int(steps)
    assert context.shape == (B, H) and z.shape == (B, LAT)

    nc = _get_nc(steps)
    in_maps = _host_prep(context, z, W_ih, W_hh, b_ih, b_hh, W_out, b_out)
    res = run_bass_kernel_spmd(nc, in_maps, core_ids=list(range(NCORES)))
    out = np.concatenate([res.results[c]["out"] for c in range(NCORES)], axis=0)
    return out



# revision 30
# speedup vs baseline: 1.2087x; 1.2087x over previous
"""Trainium2 Bass kernel for the autoregressive GRU decoder.

Problem: 512 sequential GRU steps over batch 4096, hidden 128; per step the
3-dim position output feeds back into the input.  Data-parallel over 8 cores
(512 batch rows per core), with the per-core batch split into 2 streams of
256 so the engines can pipeline across the sequential dependency chain.

Measured on hardware: 2.14 ms for 512 steps (vs 2.77 ms for the previous
revision of this kernel); per-step period ~4.1 us, bounded by the serial
dependency chain MM -> sigmoid -> mult -> add -> tanh -> sub -> mult
across TensorE/ACT/DVE with two anti-phased streams sharing the engines.

Algebraic restructuring done on the host (validated vs fp64 golden):
  - pos_k = W_out h_k + b_out feeds the next step's input, so the input
    matmul folds into the hidden one: W_c = W_ih[:, :3] @ W_out + W_hh, with
    all z / bias contributions collapsed into one constant C per gate.
  - The gate pre-activations live persistently in PSUM.  They are
    initialized once with C (via an identity matmul) + W @ h, and every
    subsequent step only accumulates W_c @ delta where
    delta = h_new - h = (u - 1) * (h - n).  No per-step constant re-adds.
  - pos is delta-accumulated directly in OUTPUT layout: posT[128(b), 4c*3i]
    persistent in one PSUM bank, where chunk c covers batch rows c*128..;
    each step 4 tiny N=3 matmuls (delta chunk stationary, W_out^T moving)
    accumulate it, and ONE [128, 12] ACT copy snapshots it (303 ns vs 677
    for the untransposed [3, 512] layout).  No final transpose pass: the
    staging buffer DMAs straight into out[b, t, i].
  - h lives in a PSUM bank, delta-accumulated by per-stream identity
    matmuls on TensorE.  This removes GpSimd from the kernel entirely:
    GpSimd shares its SBUF port with VectorE, and its streaming was
    measured to slow concurrent 2-port DVE ops by up to 2x.

Layout per core: PSUM banks: rz_A, nx_A, rz_B, nx_B (gate preacts,
[r|z] / [xn|hn] halves), h [128, 512], posT [128, 12].
"""

import os
import numpy as np

B, H, LAT, IN = 4096, 128, 32, 3
NCORES = 8
BSH = B // NCORES          # 512 batch rows per core
NST = 2                    # streams per core
BST = BSH // NST           # 256 batch rows per stream

# "f32" | "f32r" | "bf16" for the per-step delta matmuls (gates + pos);
# the init matmuls always run exact f32.
MM_DTYPE = os.environ.get("KERNEL_MM_DTYPE", "bf16")
# elementwise working dtype for t/s/n/ru/dd tiles: "f32" | "bf16"
EW_DTYPE = os.environ.get("KERNEL_EW_DTYPE", "bf16")

_CACHE = {}


def _host_prep(context, z, W_ih, W_hh, b_ih, b_hh, W_out, b_out):
    """Fold weights/constants; build per-core input maps."""
    f32 = np.float32
    sl = [slice(0, H), slice(H, 2 * H), slice(2 * H, 3 * H)]  # r, z, n rows

    Wp = (W_ih[:, :IN] @ W_out).astype(f32)           # pos feedback fold
    C0 = (W_ih[:, IN:] @ z.T + b_ih[:, None]).astype(f32)     # [384, B]
    C1 = (C0 + (W_ih[:, :IN] @ b_out)[:, None]).astype(f32)

    Wc_r = (Wp[sl[0]] + W_hh[sl[0]]).astype(f32)
    Wc_z = (Wp[sl[1]] + W_hh[sl[1]]).astype(f32)
    Wc_xn = Wp[sl[2]].astype(f32)
    W_hn = W_hh[sl[2]].astype(f32)

    def asc(a):
        return np.ascontiguousarray(a, dtype=f32)

    weights = {
        "w_r": asc(Wc_r.T), "w_z": asc(-Wc_z.T),
        "w_xn": asc(Wc_xn.T), "w_hn": asc(W_hn.T),
        "w0_r": asc(W_hh[sl[0]].T), "w0_z": asc(-W_hh[sl[1]].T),
        "w_out_t": asc(W_out.T),                       # [128, 3]
        "ident": np.eye(H, dtype=f32),
        "c_posT": asc(np.broadcast_to(np.tile(b_out, BSH // 128)[None, :],
                                      (H, (BSH // 128) * IN))),
    }

    in_maps = []
    for c in range(NCORES):
        bs = slice(c * BSH, (c + 1) * BSH)
        m = dict(weights)
        m["h0t"] = asc(context[bs].T)                  # [H, BSH]
        for X in range(NST):
            cs = slice(c * BSH + X * BST, c * BSH + (X + 1) * BST)
            bhh = b_hh[:, None]
            m[f"c0_rz_{X}"] = asc(np.concatenate(
                [C0[sl[0], cs] + bhh[sl[0]],
                 -(C0[sl[1], cs] + bhh[sl[1]])], axis=1))
            m[f"c1_rz_{X}"] = asc(np.concatenate(
                [C1[sl[0], cs] + bhh[sl[0]],
                 -(C1[sl[1], cs] + bhh[sl[1]])], axis=1))
            hn_const = np.broadcast_to(b_hh[sl[2]][:, None], (H, BST))
            m[f"c0_nx_{X}"] = asc(np.concatenate([C0[sl[2], cs], hn_const], axis=1))
            m[f"c1_nx_{X}"] = asc(np.concatenate([C1[sl[2], cs], hn_const], axis=1))
        in_maps.append(m)
    return in_maps


def _build(steps):
    import concourse.bacc as bacc
    import concourse.mybir as mybir
    from concourse.tile import TileContext

    f32 = mybir.dt.float32
    bf16 = mybir.dt.bfloat16
    Act = mybir.ActivationFunctionType
    Op = mybir.AluOpType

    ew_dt = bf16 if EW_DTYPE == "bf16" else f32
    # dtype for the per-step delta matmul operands (weights + delta).
    mm_dt = {"f32": f32, "f32r": mybir.dt.float32r, "bf16": bf16}[MM_DTYPE]

    nc = bacc.Bacc("TRN2", target_bir_lowering=False, debug=False)

    # ---- DRAM parameters ----
    names_2d = ["h0t"] + [f"c{i}_{g}_{X}" for i in (0, 1)
                          for g in ("rz", "nx") for X in range(NST)]
    params = {}
    for n in names_2d:
        params[n] = nc.declare_dram_parameter(n, [H, BSH], f32, isOutput=False)
    for n in ("w_r", "w_z", "w_xn", "w_hn", "w0_r", "w0_z", "ident"):
        params[n] = nc.declare_dram_parameter(n, [H, H], f32, isOutput=False)
    params["w_out_t"] = nc.declare_dram_parameter("w_out_t", [H, IN], f32,
                                                  isOutput=False)
    params["c_posT"] = nc.declare_dram_parameter(
        "c_posT", [H, (BSH // 128) * IN], f32, isOutput=False)
    p_out = nc.declare_dram_parameter("out", [BSH, steps, IN], f32,
                                      isOutput=True)
    NCH = BSH // 128           # batch chunks per core in the posT layout

    with TileContext(nc) as tc, \
            tc.tile_pool(name="const", bufs=1) as cpool, \
            tc.tile_pool(name="state", bufs=1) as spool, \
            tc.tile_pool(name="work", bufs=3) as wpool, \
            tc.tile_pool(name="psum", bufs=1, space="PSUM") as ppool:

        sb = {}
        for n, p in params.items():
            t = cpool.tile(list(p.shape), f32, name=f"sb_{n}")
            nc.sync.dma_start(out=t[:], in_=p[:])
            sb[n] = t

        # per-step matmul weights, converted to the mm dtype once
        stepw = {}
        for n in ("w_r", "w_z", "w_xn", "w_hn", "w_out_t", "ident"):
            if MM_DTYPE != "f32":
                t = cpool.tile(list(params[n].shape), mm_dt, name=f"cw_{n}")
                nc.vector.tensor_copy(t[:], sb[n][:])
                stepw[n] = t
            else:
                stepw[n] = sb[n]

        h = spool.tile([H, BSH], f32, name="h")
        nc.sync.dma_start(out=h[:], in_=params["h0t"][:])

        rz = [ppool.tile([128, 2 * BST], f32, name=f"ps_rz{X}")
              for X in range(NST)]
        nx = [ppool.tile([128, 2 * BST], f32, name=f"ps_nx{X}")
              for X in range(NST)]
        # hidden state, PSUM-resident, delta-accumulated by TensorE
        hps = ppool.tile([128, BSH], f32, name="ps_h")
        nc.tensor.matmul(hps[:], sb["ident"][:], h[:], start=True, stop=True)
        # scratch bank + bf16 source for PE warm-up filler matmuls: keep the
        # HAM activity monitor above its busy threshold so the PE clock gate
        # holds K=8/8 (2.4 GHz) instead of throttling to 4/8 (1.2 GHz).
        scratch = ppool.tile([128, 512], f32, name="ps_scratch")
        dummy_src = cpool.tile([128, 512], mm_dt, name="dummy_src")
        nc.vector.tensor_copy(dummy_src[:], sb["h0t"][:])

        def emit_filler():
            nc.tensor.matmul(scratch[:], stepw["w_r"][:], dummy_src[:],
                             start=True, stop=True)

        # pos in transposed/output layout: row p of chunk c is batch row
        # c*128+p; cols (c, i) hold pos[i].  Init: b_out + W_out h_0 (f32).
        posT = ppool.tile([128, NCH * IN], f32, name="ps_posT")
        # ONE start=True matmul for the whole region: start clears the
        # bank's has_written bits bank-wide, so per-chunk start=True inits
        # would wipe the previous chunks' accumulate state.
        nc.tensor.matmul(posT[:], sb["ident"][:], sb["c_posT"][:],
                         start=True, stop=False)
        for c in range(NCH):
            nc.tensor.matmul(posT[:, c * IN:(c + 1) * IN],
                             sb["h0t"][:, c * 128:(c + 1) * 128],
                             sb["w_out_t"][:], start=False, stop=True)

        delta_prev = [None, None]
        stagger = {}

        def emit_pos(dja):
            """posT += (delta chunk)^T @ W_out^T: 4 tiny N=3 matmuls with the
            delta chunk as the stationary operand; off the recurrence path."""
            for c in range(NCH):
                nc.tensor.matmul(posT[:, c * IN:(c + 1) * IN],
                                 dja[:, c * 128:(c + 1) * 128],
                                 stepw["w_out_t"][:], start=False, stop=True)

        TSNAP = 16                 # pos snapshots buffered in SBUF per DMA
        W = NCH * IN               # snapshot width (12 cols)
        stg = [None]

        def emit_snap(k):
            """Snapshot posT (state of step k) into the SBUF staging buffer
            (one cheap [128, 12] ACT copy, explicitly ordered after the
            chain-critical tanh so it never head-of-line blocks the ACT
            FIFO) and DMA a TSNAP-step block straight to out[b, t, i]."""
            kloc = k % TSNAP
            if kloc == 0:
                stg[0] = wpool.tile([128, TSNAP * W], f32, name="stg", bufs=2)
            # stg is laid out (c, t, i) so the flush DMA's DRAM AP fuses
            # (t i) into one contiguous dim (3-dim DMA limit).
            cp = nc.scalar.copy(
                stg[0][:].rearrange("p (c t i) -> p c t i",
                                    c=NCH, t=TSNAP)[:, :, kloc, :],
                posT[:].rearrange("p (c i) -> p c i", c=NCH))
            lt = stagger.get("last_tanh")
            if lt is not None:
                add_dep_helper(cp.ins, lt.ins, reason="snap after tanh")
            if kloc == TSNAP - 1:
                t0 = k - kloc
                nc.sync.dma_start(
                    out=p_out[:, t0:t0 + TSNAP, :].rearrange(
                        "(c p) t i -> p c (t i)", c=NCH),
                    in_=stg[0][:].rearrange("p (c ti) -> p c ti", c=NCH,
                                            ti=TSNAP * IN))

        from concourse.tile_rust import add_dep_helper

        def emit_gates(k, mode, X):
            """Gate matmuls + pos for one stream; returns the first matmul
            (for the anti-phase dependency ladder)."""
            first = None
            if mode < 2:
                ci = f"c{mode}"
                first = nc.tensor.matmul(rz[X][:], sb["ident"][:],
                                         sb[f"{ci}_rz_{X}"][:],
                                         start=True, stop=False)
                nc.tensor.matmul(nx[X][:], sb["ident"][:],
                                 sb[f"{ci}_nx_{X}"][:],
                                 start=True, stop=(mode == 0))
                wr = sb["w0_r"] if mode == 0 else sb["w_r"]
                wz = sb["w0_z"] if mode == 0 else sb["w_z"]
                gate_mms = [(wr, rz, 0, False), (wz, rz, 1, True),
                            (sb["w_hn"], nx, 1, True)]
                if mode == 1:
                    gate_mms.insert(2, (sb["w_xn"], nx, 0, False))
                for w, bank, half, stop in gate_mms:
                    nc.tensor.matmul(
                        bank[X][:, half * BST:(half + 1) * BST],
                        w[:], h[:, X * BST:(X + 1) * BST],
                        start=False, stop=stop)
            else:
                for w, bank, half, stop in [
                        (stepw["w_r"], rz, 0, False),
                        (stepw["w_z"], rz, 1, True),
                        (stepw["w_xn"], nx, 0, False),
                        (stepw["w_hn"], nx, 1, True)]:
                    mm = nc.tensor.matmul(
                        bank[X][:, half * BST:(half + 1) * BST],
                        w[:], delta_prev[X][:],
                        start=False, stop=stop)
                    if first is None:
                        first = mm
            return first

        for k in range(steps):
            mode = 0 if k == 0 else (1 if k == 1 else 2)

            # Anti-phase ladder: emit [gates_A, sigma_A, gates_B, sigma_B]
            # with gates_B additionally depending on sigma_A (same step) and
            # gates_A on sigma_B (previous step).  Without this the two
            # stream pipelines bunch up in phase and serialize on the shared
            # engines (bus-bunching); the ladder pins a half-step offset.
            ru = [wpool.tile([128, 2 * BST], ew_dt, name=f"ru{X}", bufs=4)
                  for X in range(NST)]
            tt = [wpool.tile([128, BST], ew_dt, name=f"t{X}", bufs=4)
                  for X in range(NST)]
            ts = [wpool.tile([128, BST], ew_dt, name=f"s{X}", bufs=4)
                  for X in range(NST)]
            old_delta = delta_prev
            dj = wpool.tile([128, BSH], mm_dt, name="dj", bufs=4)
            new_delta = [dj[:, 0:BST], dj[:, BST:2 * BST], dj]
            for X in range(NST):
                # ---- full per-stream chain: every engine's FIFO sees
                # [A-op, B-op, A-op, ...] so anti-phased streams never
                # head-of-line block each other. ----
                mm_first = emit_gates(k, mode, X)
                if mm_first is not None:
                    other = stagger.get("s_prev" if X == 0 else "s_cur")
                    if other is not None:
                        add_dep_helper(mm_first.ins, other.ins,
                                       reason="anti-phase ladder")
                # h <- h + delta(k-1) for this stream, as an identity-matmul
                # accumulate into the PSUM h bank.  Emitted here (one step
                # late, right behind this stream's gate matmuls) so it fills
                # the PE gap before the partner stream's gates become ready
                # and never head-of-line blocks a chain matmul.
                if old_delta[0] is not None:
                    nc.tensor.matmul(hps[:, X * BST:(X + 1) * BST],
                                     stepw["ident"][:], old_delta[X],
                                     start=False, stop=True)
                sig = nc.scalar.activation(ru[X][:], rz[X][:], Act.Sigmoid)
                stagger["s_cur" if X == 0 else "s_prev"] = sig
                nc.vector.tensor_tensor(tt[X][:], nx[X][:, BST:2 * BST],
                                        ru[X][:, 0:BST], Op.mult)
                nc.vector.tensor_tensor(ts[X][:], tt[X][:],
                                        nx[X][:, 0:BST], Op.add)
                n_ = wpool.tile([128, BST], ew_dt, name=f"n{X}", bufs=4)
                stagger["last_tanh"] = nc.scalar.activation(
                    n_[:], ts[X][:], Act.Tanh)
                dd = wpool.tile([128, BST], ew_dt, name=f"dd{X}", bufs=4)
                nc.vector.tensor_tensor(dd[:], n_[:],
                                        hps[:, X * BST:(X + 1) * BST],
                                        Op.subtract)
                delta_inst = nc.vector.tensor_tensor(new_delta[X],
                                                     ru[X][:, BST:2 * BST],
                                                     dd[:], Op.mult)
                stagger["last_delta"] = delta_inst
                if X == 0:
                    # PE-idle window: stream A's gates are done and stream
                    # B's aren't ready yet (ladder) — filler can't block.
                    emit_filler()
            delta_prev = new_delta

            if k == 0:
                # SBUF h copy is still the matmul rhs for the k=1 bootstrap
                # step; bring it to h_1 once (DVE, off-chain).
                nc.vector.tensor_tensor(h[:], h[:], new_delta[2][:], Op.add)

            # pos matmuls + snapshot for the previous step (off-chain tail)
            if old_delta[0] is not None:
                emit_pos(old_delta[2])
                emit_snap(k - 1)
            emit_filler()

        # trailing pos output for the final step
        emit_pos(delta_prev[2])
        emit_snap(steps - 1)

    nc.finalize()
    return nc


def _get_nc(steps):
    key = (steps, MM_DTYPE, EW_DTYPE)
    if key not in _CACHE:
        _CACHE[key] = _build(steps)
    return _CACHE[key]


def kernel(context, z, steps, W_ih, W_hh, b_ih, b_hh, W_out, b_out):
    from concourse.bass_utils import run_bass_kernel_spmd

    context = np.asarray(context, dtype=np.float32)
    z = np.asarray(z, dtype=np.float32)
    W_ih = np.asarray(W_ih, dtype=np.float32)
    W_hh = np.asarray(W_hh, dtype=np.float32)
    b_ih = np.asarray(b_ih, dtype=np.float32)
    b_hh = np.asarray(b_hh, dtype=np.float32)
    W_out = np.asarray(W_out, dtype=np.float32)
    b_out = np.asarray(b_out, dtype=np.float32)
    steps = int(steps)
    assert context.shape == (B, H) and z.shape == (B, LAT)

    nc = _get_nc(steps)
    in_maps = _host_prep(context, z, W_ih, W_hh, b_ih, b_hh, W_out, b_out)
    res = run_bass_kernel_spmd(nc, in_maps, core_ids=list(range(NCORES)))
    out = np.concatenate([res.results[c]["out"] for c in range(NCORES)], axis=0)
    return out



# revision 36
# speedup vs baseline: 1.2450x; 1.0300x over previous
"""Trainium2 Bass kernel for the autoregressive GRU decoder.

Problem: 512 sequential GRU steps over batch 4096, hidden 128; per step the
3-dim position output feeds back into the input.  Data-parallel over 8 cores
(512 batch rows per core), with the per-core batch split into 2 streams of
256 so the engines can pipeline across the sequential dependency chain.

Measured on hardware: 2.14 ms for 512 steps (vs 2.77 ms for the previous
revision of this kernel); per-step period ~4.1 us, bounded by the serial
dependency chain MM -> sigmoid -> mult -> add -> tanh -> sub -> mult
across TensorE/ACT/DVE with two anti-phased streams sharing the engines.

Algebraic restructuring done on the host (validated vs fp64 golden):
  - pos_k = W_out h_k + b_out feeds the next step's input, so the input
    matmul folds into the hidden one: W_c = W_ih[:, :3] @ W_out + W_hh, with
    all z / bias contributions collapsed into one constant C per gate.
  - The gate pre-activations live persistently in PSUM.  They are
    initialized once with C (via an identity matmul) + W @ h, and every
    subsequent step only accumulates W_c @ delta where
    delta = h_new - h = (u - 1) * (h - n).  No per-step constant re-adds.
  - pos is delta-accumulated directly in OUTPUT layout: posT[128(b), 4c*3i]
    persistent in one PSUM bank, where chunk c covers batch rows c*128..;
    each step 4 tiny N=3 matmuls (delta chunk stationary, W_out^T moving)
    accumulate it, and ONE [128, 12] ACT copy snapshots it (303 ns vs 677
    for the untransposed [3, 512] layout).  No final transpose pass: the
    staging buffer DMAs straight into out[b, t, i].
  - h lives in a PSUM bank, delta-accumulated by per-stream identity
    matmuls on TensorE.  This removes GpSimd from the kernel entirely:
    GpSimd shares its SBUF port with VectorE, and its streaming was
    measured to slow concurrent 2-port DVE ops by up to 2x.

Layout per core: PSUM banks: rz_A, nx_A, rz_B, nx_B (gate preacts,
[r|z] / [xn|hn] halves), h [128, 512], posT [128, 12].
"""

import os
import numpy as np

B, H, LAT, IN = 4096, 128, 32, 3
NCORES = 8
BSH = B // NCORES          # 512 batch rows per core
NST = 2                    # streams per core
BST = BSH // NST           # 256 batch rows per stream

# "f32" | "f32r" | "bf16" for the per-step delta matmuls (gates + pos);
# the init matmuls always run exact f32.
MM_DTYPE = os.environ.get("KERNEL_MM_DTYPE", "bf16")
# elementwise working dtype for t/s/n/ru/dd tiles: "f32" | "bf16"
EW_DTYPE = os.environ.get("KERNEL_EW_DTYPE", "bf16")

_CACHE = {}


def _host_prep(context, z, W_ih, W_hh, b_ih, b_hh, W_out, b_out):
    """Fold weights/constants; build per-core input maps."""
    f32 = np.float32
    sl = [slice(0, H), slice(H, 2 * H), slice(2 * H, 3 * H)]  # r, z, n rows

    Wp = (W_ih[:, :IN] @ W_out).astype(f32)           # pos feedback fold
    C0 = (W_ih[:, IN:] @ z.T + b_ih[:, None]).astype(f32)     # [384, B]
    C1 = (C0 + (W_ih[:, :IN] @ b_out)[:, None]).astype(f32)

    Wc_r = (Wp[sl[0]] + W_hh[sl[0]]).astype(f32)
    Wc_z = (Wp[sl[1]] + W_hh[sl[1]]).astype(f32)
    Wc_xn = Wp[sl[2]].astype(f32)
    W_hn = W_hh[sl[2]].astype(f32)

    def asc(a):
        return np.ascontiguousarray(a, dtype=f32)

    weights = {
        "w_r": asc(Wc_r.T), "w_z": asc(-Wc_z.T),
        "w_xn": asc(Wc_xn.T), "w_hn": asc(W_hn.T),
        "w0_r": asc(W_hh[sl[0]].T), "w0_z": asc(-W_hh[sl[1]].T),
        "w_out_t": asc(W_out.T),                       # [128, 3]
        "ident": np.eye(H, dtype=f32),
        "c_posT": asc(np.broadcast_to(np.tile(b_out, BSH // 128)[None, :],
                                      (H, (BSH // 128) * IN))),
    }

    in_maps = []
    for c in range(NCORES):
        bs = slice(c * BSH, (c + 1) * BSH)
        m = dict(weights)
        m["h0t"] = asc(context[bs].T)                  # [H, BSH]
        for X in range(NST):
            cs = slice(c * BSH + X * BST, c * BSH + (X + 1) * BST)
            bhh = b_hh[:, None]
            m[f"c0_rz_{X}"] = asc(np.concatenate(
                [C0[sl[0], cs] + bhh[sl[0]],
                 -(C0[sl[1], cs] + bhh[sl[1]])], axis=1))
            m[f"c1_rz_{X}"] = asc(np.concatenate(
                [C1[sl[0], cs] + bhh[sl[0]],
                 -(C1[sl[1], cs] + bhh[sl[1]])], axis=1))
            hn_const = np.broadcast_to(b_hh[sl[2]][:, None], (H, BST))
            m[f"c0_nx_{X}"] = asc(np.concatenate([C0[sl[2], cs], hn_const], axis=1))
            m[f"c1_nx_{X}"] = asc(np.concatenate([C1[sl[2], cs], hn_const], axis=1))
        in_maps.append(m)
    return in_maps


def _build(steps):
    import concourse.bacc as bacc
    import concourse.mybir as mybir
    from concourse.tile import TileContext

    f32 = mybir.dt.float32
    bf16 = mybir.dt.bfloat16
    Act = mybir.ActivationFunctionType
    Op = mybir.AluOpType

    ew_dt = bf16 if EW_DTYPE == "bf16" else f32
    # dtype for the per-step delta matmul operands (weights + delta).
    mm_dt = {"f32": f32, "f32r": mybir.dt.float32r, "bf16": bf16}[MM_DTYPE]

    nc = bacc.Bacc("TRN2", target_bir_lowering=False, debug=False)

    # ---- DRAM parameters ----
    names_2d = ["h0t"] + [f"c{i}_{g}_{X}" for i in (0, 1)
                          for g in ("rz", "nx") for X in range(NST)]
    params = {}
    for n in names_2d:
        params[n] = nc.declare_dram_parameter(n, [H, BSH], f32, isOutput=False)
    for n in ("w_r", "w_z", "w_xn", "w_hn", "w0_r", "w0_z", "ident"):
        params[n] = nc.declare_dram_parameter(n, [H, H], f32, isOutput=False)
    params["w_out_t"] = nc.declare_dram_parameter("w_out_t", [H, IN], f32,
                                                  isOutput=False)
    params["c_posT"] = nc.declare_dram_parameter(
        "c_posT", [H, (BSH // 128) * IN], f32, isOutput=False)
    p_out = nc.declare_dram_parameter("out", [BSH, steps, IN], f32,
                                      isOutput=True)
    NCH = BSH // 128           # batch chunks per core in the posT layout

    with TileContext(nc) as tc, \
            tc.tile_pool(name="const", bufs=1) as cpool, \
            tc.tile_pool(name="state", bufs=1) as spool, \
            tc.tile_pool(name="work", bufs=3) as wpool, \
            tc.tile_pool(name="psum", bufs=1, space="PSUM") as ppool:

        sb = {}
        for n, p in params.items():
            t = cpool.tile(list(p.shape), f32, name=f"sb_{n}")
            nc.sync.dma_start(out=t[:], in_=p[:])
            sb[n] = t

        # per-step matmul weights, converted to the mm dtype once
        stepw = {}
        for n in ("w_r", "w_z", "w_xn", "w_hn", "w_out_t", "ident"):
            if MM_DTYPE != "f32":
                t = cpool.tile(list(params[n].shape), mm_dt, name=f"cw_{n}")
                nc.vector.tensor_copy(t[:], sb[n][:])
                stepw[n] = t
            else:
                stepw[n] = sb[n]

        h = spool.tile([H, BSH], f32, name="h")
        nc.sync.dma_start(out=h[:], in_=params["h0t"][:])

        # r and z preacts live in SEPARATE PSUM tiles: PSUM dependency
        # tracking is tile-granular, so with a joint [r|z] tile the r-half
        # sigmoid inherits a false dependency on the w_z matmul and the
        # chain head becomes two serial matmuls instead of one.
        rr = [ppool.tile([128, BST], f32, name=f"ps_r{X}")
              for X in range(NST)]
        zz = [ppool.tile([128, BST], f32, name=f"ps_z{X}")
              for X in range(NST)]
        nx = [ppool.tile([128, 2 * BST], f32, name=f"ps_nx{X}")
              for X in range(NST)]
        # hidden state, PSUM-resident, delta-accumulated by TensorE
        hps = ppool.tile([128, BSH], f32, name="ps_h")
        nc.tensor.matmul(hps[:], sb["ident"][:], h[:], start=True, stop=True)
        # pos in transposed/output layout: row p of chunk c is batch row
        # c*128+p; cols (c, i) hold pos[i].  Init: b_out + W_out h_0 (f32).
        posT = ppool.tile([128, NCH * IN], f32, name="ps_posT")
        # ONE start=True matmul for the whole region: start clears the
        # bank's has_written bits bank-wide, so per-chunk start=True inits
        # would wipe the previous chunks' accumulate state.
        nc.tensor.matmul(posT[:], sb["ident"][:], sb["c_posT"][:],
                         start=True, stop=False)
        for c in range(NCH):
            nc.tensor.matmul(posT[:, c * IN:(c + 1) * IN],
                             sb["h0t"][:, c * 128:(c + 1) * 128],
                             sb["w_out_t"][:], start=False, stop=True)

        delta_prev = [None, None]
        stagger = {}

        def emit_pos(dja):
            """posT += (delta chunk)^T @ W_out^T: 4 tiny N=3 matmuls with the
            delta chunk as the stationary operand; off the recurrence path."""
            for c in range(NCH):
                nc.tensor.matmul(posT[:, c * IN:(c + 1) * IN],
                                 dja[:, c * 128:(c + 1) * 128],
                                 stepw["w_out_t"][:], start=False, stop=True)

        TSNAP = 16                 # pos snapshots buffered in SBUF per DMA
        W = NCH * IN               # snapshot width (12 cols)
        stg = [None]

        def emit_snap(k):
            """Snapshot posT (state of step k) into the SBUF staging buffer
            (one cheap [128, 12] ACT copy, explicitly ordered after the
            chain-critical tanh so it never head-of-line blocks the ACT
            FIFO) and DMA a TSNAP-step block straight to out[b, t, i]."""
            kloc = k % TSNAP
            if kloc == 0:
                stg[0] = wpool.tile([128, TSNAP * W], f32, name="stg", bufs=2)
            # stg is laid out (c, t, i) so the flush DMA's DRAM AP fuses
            # (t i) into one contiguous dim (3-dim DMA limit).
            cp = nc.scalar.copy(
                stg[0][:].rearrange("p (c t i) -> p c t i",
                                    c=NCH, t=TSNAP)[:, :, kloc, :],
                posT[:].rearrange("p (c i) -> p c i", c=NCH))
            lt = stagger.get("last_tanh")
            if lt is not None:
                add_dep_helper(cp.ins, lt.ins, reason="snap after tanh")
            if kloc == TSNAP - 1:
                t0 = k - kloc
                nc.sync.dma_start(
                    out=p_out[:, t0:t0 + TSNAP, :].rearrange(
                        "(c p) t i -> p c (t i)", c=NCH),
                    in_=stg[0][:].rearrange("p (c ti) -> p c ti", c=NCH,
                                            ti=TSNAP * IN))

        from concourse.tile_rust import add_dep_helper

        def emit_gates(k, mode, X):
            """Gate matmuls + pos for one stream; returns the first matmul
            (for the anti-phase dependency ladder)."""
            first = None
            if mode < 2:
                ci = f"c{mode}"
                first = nc.tensor.matmul(rr[X][:], sb["ident"][:],
                                         sb[f"{ci}_rz_{X}"][:, 0:BST],
                                         start=True, stop=False)
                nc.tensor.matmul(zz[X][:], sb["ident"][:],
                                 sb[f"{ci}_rz_{X}"][:, BST:2 * BST],
                                 start=True, stop=False)
                nc.tensor.matmul(nx[X][:], sb["ident"][:],
                                 sb[f"{ci}_nx_{X}"][:],
                                 start=True, stop=(mode == 0))
                wr = sb["w0_r"] if mode == 0 else sb["w_r"]
                wz = sb["w0_z"] if mode == 0 else sb["w_z"]
                gate_mms = [(wr, rr[X][:], True), (wz, zz[X][:], True),
                            (sb["w_hn"], nx[X][:, BST:2 * BST], True)]
                if mode == 1:
                    gate_mms.insert(2, (sb["w_xn"], nx[X][:, 0:BST], True))
                for w, out_ap, stop in gate_mms:
                    nc.tensor.matmul(out_ap, w[:],
                                     h[:, X * BST:(X + 1) * BST],
                                     start=False, stop=stop)
            else:
                # w_r first: the r-sigmoid (chain head) depends only on it.
                # w_hn next (tt's input), then w_xn (ts), w_z (u') last.
                for w, out_ap, stop in [
                        (stepw["w_r"], rr[X][:], True),
                        (stepw["w_hn"], nx[X][:, BST:2 * BST], True),
                        (stepw["w_xn"], nx[X][:, 0:BST], True),
                        (stepw["w_z"], zz[X][:], True)]:
                    mm = nc.tensor.matmul(out_ap, w[:], delta_prev[X][:],
                                          start=False, stop=stop)
                    if first is None:
                        first = mm
            return first

        for k in range(steps):
            mode = 0 if k == 0 else (1 if k == 1 else 2)

            # Anti-phase ladder: emit [gates_A, sigma_A, gates_B, sigma_B]
            # with gates_B additionally depending on sigma_A (same step) and
            # gates_A on sigma_B (previous step).  Without this the two
            # stream pipelines bunch up in phase and serialize on the shared
            # engines (bus-bunching); the ladder pins a half-step offset.
            ru = [wpool.tile([128, 2 * BST], ew_dt, name=f"ru{X}", bufs=4)
                  for X in range(NST)]
            tt = [wpool.tile([128, BST], ew_dt, name=f"t{X}", bufs=4)
                  for X in range(NST)]
            ts = [wpool.tile([128, BST], ew_dt, name=f"s{X}", bufs=4)
                  for X in range(NST)]
            old_delta = delta_prev
            dj = wpool.tile([128, BSH], mm_dt, name="dj", bufs=4)
            new_delta = [dj[:, 0:BST], dj[:, BST:2 * BST], dj]
            for X in range(NST):
                # ---- full per-stream chain: every engine's FIFO sees
                # [A-op, B-op, A-op, ...] so anti-phased streams never
                # head-of-line block each other. ----
                mm_first = emit_gates(k, mode, X)
                if mm_first is not None:
                    other = stagger.get("s_prev" if X == 0 else "s_cur")
                    if other is not None:
                        add_dep_helper(mm_first.ins, other.ins,
                                       reason="anti-phase ladder")
                # h <- h + delta(k-1) for this stream, as an identity-matmul
                # accumulate into the PSUM h bank.  Emitted here (one step
                # late, right behind this stream's gate matmuls) so it fills
                # the PE gap before the partner stream's gates become ready
                # and never head-of-line blocks a chain matmul.
                if old_delta[0] is not None:
                    nc.tensor.matmul(hps[:, X * BST:(X + 1) * BST],
                                     stepw["ident"][:], old_delta[X],
                                     start=False, stop=True)
                # split sigmoid: the r half is chain-critical (feeds tt right
                # away); the z half (u') is only needed by delta ~1.3us
                # later and fills the ACT gap while DVE runs tt/ts.
                sig = nc.scalar.activation(ru[X][:, 0:BST], rr[X][:],
                                           Act.Sigmoid)
                stagger["s_cur" if X == 0 else "s_prev"] = sig
                nc.vector.tensor_tensor(tt[X][:], nx[X][:, BST:2 * BST],
                                        ru[X][:, 0:BST], Op.mult)
                nc.vector.tensor_tensor(ts[X][:], tt[X][:],
                                        nx[X][:, 0:BST], Op.add)
                nc.scalar.activation(ru[X][:, BST:2 * BST], zz[X][:],
                                     Act.Sigmoid)
                n_ = wpool.tile([128, BST], ew_dt, name=f"n{X}", bufs=4)
                stagger["last_tanh"] = nc.scalar.activation(
                    n_[:], ts[X][:], Act.Tanh)
                dd = wpool.tile([128, BST], ew_dt, name=f"dd{X}", bufs=4)
                nc.vector.tensor_tensor(dd[:], n_[:],
                                        hps[:, X * BST:(X + 1) * BST],
                                        Op.subtract)
                delta_inst = nc.vector.tensor_tensor(new_delta[X],
                                                     ru[X][:, BST:2 * BST],
                                                     dd[:], Op.mult)
                stagger["last_delta"] = delta_inst
            delta_prev = new_delta

            if k == 0:
                # SBUF h copy is still the matmul rhs for the k=1 bootstrap
                # step; bring it to h_1 once (DVE, off-chain).
                nc.vector.tensor_tensor(h[:], h[:], new_delta[2][:], Op.add)

            # pos matmuls + snapshot for the previous step (off-chain tail)
            if old_delta[0] is not None:
                emit_pos(old_delta[2])
                emit_snap(k - 1)

        # trailing pos output for the final step
        emit_pos(delta_prev[2])
        emit_snap(steps - 1)

    nc.finalize()
    return nc


def _get_nc(steps):
    key = (steps, MM_DTYPE, EW_DTYPE)
    if key not in _CACHE:
        _CACHE[key] = _build(steps)
    return _CACHE[key]


def kernel(context, z, steps, W_ih, W_hh, b_ih, b_hh, W_out, b_out):
    from concourse.bass_utils import run_bass_kernel_spmd

    context = np.asarray(context, dtype=np.float32)
    z = np.asarray(z, dtype=np.float32)
    W_ih = np.asarray(W_ih, dtype=np.float32)
    W_hh = np.asarray(W_hh, dtype=np.float32)
    b_ih = np.asarray(b_ih, dtype=np.float32)
    b_hh = np.asarray(b_hh, dtype=np.float32)
    W_out = np.asarray(W_out, dtype=np.float32)
    b_out = np.asarray(b_out, dtype=np.float32)
    steps = int(steps)
    assert context.shape == (B, H) and z.shape == (B, LAT)

    nc = _get_nc(steps)
    in_maps = _host_prep(context, z, W_ih, W_hh, b_ih, b_hh, W_out, b_out)
    res = run_bass_kernel_spmd(nc, in_maps, core_ids=list(range(NCORES)))
    out = np.concatenate([res.results[c]["out"] for c in range(NCORES)], axis=0)
    return out

